# revision 6
# baseline (speedup 1.0000x reference)
"""Distributed column-sum-of-squares loss kernel for TRN2 (8 NeuronCores).

Computes 0.001 * || (D^T @ D) * I - I ||_F for D [262144, 512] f32, i.e.
    loss = 0.001 * sqrt( sum_j (||D[:, j]||^2 - 1)^2 )

Strategy (data parallel over rows, per the sharding hint):
  - The f32 variant of this kernel streams 64 MiB/core and sits at the
    ~358 GB/s per-NeuronCore HBM roofline (~186 us).  The loss tolerance
    admits bf16: casting D on host shifts the loss by ~1e-5 relative,
    and halves per-core HBM traffic to 32 MiB (~94 us DMA floor).
  - Shard D row-wise across the 8 cores (32768 rows each), cast bf16.
  - Per core: stream 1 MiB [128, 8*512] bf16 chunks from HBM
    (alternating the two HWDGE rings so DMA fixed costs overlap),
    square on VectorE (bf16 in/out hits the 2x DVE mode: ~68 us for the
    full shard, hidden under DMA), reduce the partition axis with a
    ones-vector bf16 matmul on TensorE (~55 us, also hidden)
    accumulating into a [1, 512] f32 PSUM bank.
  - Each core emits its partial per-column sum of squares [1, 512]; the
    tiny cross-core reduction + norm epilogue runs on host (the [d]
    vector combine the hint's all-reduce would do on-device).
"""

from contextlib import ExitStack

import ml_dtypes
import numpy as np

import concourse.bass as bass
import concourse.tile as tile
from concourse import bacc, mybir
from concourse.bass_utils import run_bass_kernel_spmd

N_CORES = 8
N_ROWS, N_COLS = 262144, 512
ROWS_PER_CORE = N_ROWS // N_CORES  # 32768
P = 128  # SBUF partitions
T = 8  # row-blocks of 128 per chunk -> free dim T*N_COLS = 4096 (1 MiB bf16)
S = ROWS_PER_CORE // (P * T)  # chunks per core

BF16 = mybir.dt.bfloat16

_NC_CACHE = {}


def _build_nc():
    nc = bacc.Bacc(
        "TRN2", target_bir_lowering=False, debug=False, num_devices=N_CORES
    )
    d_in = nc.dram_tensor(
        "d_shard", [ROWS_PER_CORE, N_COLS], BF16, kind="ExternalInput"
    ).ap()
    out = nc.dram_tensor(
        "partial", [1, N_COLS], mybir.dt.float32, kind="ExternalOutput"
    ).ap()

    # [S, 128, T, 512]; partition p reads a contiguous T*512-elem (8 KiB) run
    view = d_in.rearrange("(s p t) d -> s p t d", p=P, t=T)

    with tile.TileContext(nc) as tc, ExitStack() as ctx:
        in_pool = ctx.enter_context(tc.tile_pool(name="in", bufs=6))
        sq_pool = ctx.enter_context(tc.tile_pool(name="sq", bufs=3))
        psum_pool = ctx.enter_context(tc.tile_pool(name="psum", bufs=1, space="PSUM"))
        const_pool = ctx.enter_context(tc.tile_pool(name="const", bufs=1))
        res_pool = ctx.enter_context(tc.tile_pool(name="res", bufs=1))

        ones = const_pool.tile([P, 1], BF16)
        nc.vector.memset(ones, 1.0)
        psum = psum_pool.tile([1, N_COLS], mybir.dt.float32)

        for s in range(S):
            t_in = in_pool.tile([P, T, N_COLS], BF16)
            # alternate the two HWDGE rings so per-DMA fixed costs overlap
            dma_eng = nc.sync if s % 2 == 0 else nc.scalar
            dma_eng.dma_start(out=t_in, in_=view[s])
            sq = sq_pool.tile([P, T, N_COLS], BF16)
            # square on DVE (bf16 in/out -> 2x mode) in halves so the matmuls
            # of the first half overlap the second half's square
            H = T // 2
            for h in range(2):
                hs = slice(h * H, (h + 1) * H)
                nc.vector.tensor_mul(sq[:, hs, :], t_in[:, hs, :], t_in[:, hs, :])
                for t in range(h * H, (h + 1) * H):
                    # psum[1, 512] += ones[128, 1].T @ sq[:, t, :]
                    nc.tensor.matmul(
                        psum,
                        lhsT=ones,
                        rhs=sq[:, t, :],
                        start=(s == 0 and t == 0),
                        stop=(s == S - 1 and t == T - 1),
                    )

        res = res_pool.tile([1, N_COLS], mybir.dt.float32)
        nc.vector.tensor_copy(res, psum)
        nc.sync.dma_start(out=out, in_=res)

    nc.compile()
    return nc


def _run_device(D, **spmd_kwargs):
    """Run the per-core partial reduction; returns (partials [8, 512], results)."""
    if "nc" not in _NC_CACHE:
        _NC_CACHE["nc"] = _build_nc()
    nc = _NC_CACHE["nc"]
    D16 = np.asarray(D, dtype=np.float32).astype(ml_dtypes.bfloat16)
    shards = np.split(D16, N_CORES, axis=0)
    in_maps = [{"d_shard": np.ascontiguousarray(s)} for s in shards]
    res = run_bass_kernel_spmd(nc, in_maps, core_ids=list(range(N_CORES)), **spmd_kwargs)
    partials = np.stack([np.asarray(r["partial"]).reshape(N_COLS) for r in res.results])
    return partials, res


def kernel(D):
    partials, _ = _run_device(D)
    total = partials.sum(axis=0, dtype=np.float64)
    resid = total - 1.0
    loss = 0.001 * np.sqrt(np.sum(resid * resid))
    return np.array(loss, dtype=np.float32)


# revision 8
# speedup vs baseline: 1.1312x; 1.1312x over previous
"""Distributed column-sum-of-squares loss kernel for TRN2 (8 NeuronCores).

Computes 0.001 * || (D^T @ D) * I - I ||_F for D [262144, 512] f32, i.e.
    loss = 0.001 * sqrt( sum_j (||D[:, j]||^2 - 1)^2 )

Strategy (data parallel over rows, per the sharding hint):
  - The f32 variant of this kernel streams 64 MiB/core and sits at the
    ~358 GB/s per-NeuronCore HBM roofline (~186 us).  The loss tolerance
    admits bf16: casting D on host shifts the loss by ~1e-5 relative,
    and halves per-core HBM traffic to 32 MiB (~94 us DMA floor).
  - Shard D row-wise across the 8 cores (32768 rows each), cast bf16.
  - Per core: stream 1 MiB [128, 8*512] bf16 chunks from HBM
    (alternating the two HWDGE rings so DMA fixed costs overlap),
    square on VectorE (bf16 in/out hits the 2x DVE mode: ~68 us for the
    full shard, hidden under DMA), reduce the partition axis with a
    ones-vector bf16 matmul on TensorE (~55 us, also hidden)
    accumulating into a [1, 512] f32 PSUM bank.
  - Each core emits its partial per-column sum of squares [1, 512]; the
    tiny cross-core reduction + norm epilogue runs on host (the [d]
    vector combine the hint's all-reduce would do on-device).
"""

from contextlib import ExitStack

import ml_dtypes
import numpy as np

import concourse.bass as bass
import concourse.tile as tile
from concourse import bacc, mybir
from concourse.bass_utils import run_bass_kernel_spmd

N_CORES = 8
N_ROWS, N_COLS = 262144, 512
ROWS_PER_CORE = N_ROWS // N_CORES  # 32768
P = 128  # SBUF partitions
T = 16  # row-blocks of 128 per chunk -> free dim T*N_COLS = 8192 (2 MiB bf16)
S = ROWS_PER_CORE // (P * T)  # chunks per core

BF16 = mybir.dt.bfloat16

_NC_CACHE = {}


def _build_nc():
    nc = bacc.Bacc(
        "TRN2", target_bir_lowering=False, debug=False, num_devices=N_CORES
    )
    d_in = nc.dram_tensor(
        "d_shard", [ROWS_PER_CORE, N_COLS], BF16, kind="ExternalInput"
    ).ap()
    out = nc.dram_tensor(
        "partial", [1, N_COLS], mybir.dt.float32, kind="ExternalOutput"
    ).ap()

    # [S, 128, T, 512]; partition p reads a contiguous T*512-elem (8 KiB) run
    view = d_in.rearrange("(s p t) d -> s p t d", p=P, t=T)

    with tile.TileContext(nc) as tc, ExitStack() as ctx:
        in_pool = ctx.enter_context(tc.tile_pool(name="in", bufs=4))
        sq_pool = ctx.enter_context(tc.tile_pool(name="sq", bufs=2))
        psum_pool = ctx.enter_context(tc.tile_pool(name="psum", bufs=1, space="PSUM"))
        const_pool = ctx.enter_context(tc.tile_pool(name="const", bufs=1))
        res_pool = ctx.enter_context(tc.tile_pool(name="res", bufs=1))

        ones = const_pool.tile([P, 1], BF16)
        nc.vector.memset(ones, 1.0)
        psum = psum_pool.tile([1, N_COLS], mybir.dt.float32)

        for s in range(S):
            t_in = in_pool.tile([P, T, N_COLS], BF16)
            # alternate the two HWDGE rings so per-DMA fixed costs overlap
            dma_eng = nc.sync if s % 2 == 0 else nc.scalar
            dma_eng.dma_start(out=t_in, in_=view[s])
            sq = sq_pool.tile([P, T, N_COLS], BF16)
            # square on DVE (bf16 in/out -> 2x mode) in halves so the matmuls
            # of the first half overlap the second half's square
            H = T // 2
            for h in range(2):
                hs = slice(h * H, (h + 1) * H)
                nc.vector.tensor_mul(sq[:, hs, :], t_in[:, hs, :], t_in[:, hs, :])
                for t in range(h * H, (h + 1) * H):
                    # psum[1, 512] += ones[128, 1].T @ sq[:, t, :]
                    nc.tensor.matmul(
                        psum,
                        lhsT=ones,
                        rhs=sq[:, t, :],
                        start=(s == 0 and t == 0),
                        stop=(s == S - 1 and t == T - 1),
                    )

        res = res_pool.tile([1, N_COLS], mybir.dt.float32)
        nc.vector.tensor_copy(res, psum)
        nc.sync.dma_start(out=out, in_=res)

    nc.compile()
    return nc


def _run_device(D, **spmd_kwargs):
    """Run the per-core partial reduction; returns (partials [8, 512], results)."""
    if "nc" not in _NC_CACHE:
        _NC_CACHE["nc"] = _build_nc()
    nc = _NC_CACHE["nc"]
    D16 = np.asarray(D, dtype=np.float32).astype(ml_dtypes.bfloat16)
    shards = np.split(D16, N_CORES, axis=0)
    in_maps = [{"d_shard": np.ascontiguousarray(s)} for s in shards]
    res = run_bass_kernel_spmd(nc, in_maps, core_ids=list(range(N_CORES)), **spmd_kwargs)
    partials = np.stack([np.asarray(r["partial"]).reshape(N_COLS) for r in res.results])
    return partials, res


def kernel(D):
    partials, _ = _run_device(D)
    total = partials.sum(axis=0, dtype=np.float64)
    resid = total - 1.0
    loss = 0.001 * np.sqrt(np.sum(resid * resid))
    return np.array(loss, dtype=np.float32)


# revision 9
# speedup vs baseline: 1.1334x; 1.0020x over previous
"""Distributed column-sum-of-squares loss kernel for TRN2 (8 NeuronCores).

Computes 0.001 * || (D^T @ D) * I - I ||_F for D [262144, 512] f32, i.e.
    loss = 0.001 * sqrt( sum_j (||D[:, j]||^2 - 1)^2 )

Strategy (data parallel over rows, per the sharding hint):
  - The f32 variant of this kernel streams 64 MiB/core and sits at the
    ~358 GB/s per-NeuronCore HBM roofline (~186 us).  The loss tolerance
    admits bf16: casting D on host shifts the loss by ~1e-5 relative,
    and halves per-core HBM traffic to 32 MiB (~94 us DMA floor).
  - Shard D row-wise across the 8 cores (32768 rows each), cast bf16.
  - Per core: stream 1 MiB [128, 8*512] bf16 chunks from HBM
    (alternating the two HWDGE rings so DMA fixed costs overlap),
    square on VectorE (bf16 in/out hits the 2x DVE mode: ~68 us for the
    full shard, hidden under DMA), reduce the partition axis with a
    ones-vector bf16 matmul on TensorE (~55 us, also hidden)
    accumulating into a [1, 512] f32 PSUM bank.
  - Each core emits its partial per-column sum of squares [1, 512]; the
    tiny cross-core reduction + norm epilogue runs on host (the [d]
    vector combine the hint's all-reduce would do on-device).
"""

from contextlib import ExitStack

import ml_dtypes
import numpy as np

import concourse.bass as bass
import concourse.tile as tile
from concourse import bacc, mybir
from concourse.bass_utils import run_bass_kernel_spmd

N_CORES = 8
N_ROWS, N_COLS = 262144, 512
ROWS_PER_CORE = N_ROWS // N_CORES  # 32768
P = 128  # SBUF partitions
T = 16  # row-blocks of 128 per chunk -> free dim T*N_COLS = 8192 (2 MiB bf16)
S = ROWS_PER_CORE // (P * T)  # chunks per core

BF16 = mybir.dt.bfloat16

_NC_CACHE = {}


def _build_nc():
    nc = bacc.Bacc(
        "TRN2", target_bir_lowering=False, debug=False, num_devices=N_CORES
    )
    d_in = nc.dram_tensor(
        "d_shard", [ROWS_PER_CORE, N_COLS], BF16, kind="ExternalInput"
    ).ap()
    out = nc.dram_tensor(
        "partial", [1, N_COLS], mybir.dt.float32, kind="ExternalOutput"
    ).ap()

    # [S, 128, T, 512]; partition p reads a contiguous T*512-elem (8 KiB) run
    view = d_in.rearrange("(s p t) d -> s p t d", p=P, t=T)

    with tile.TileContext(nc) as tc, ExitStack() as ctx:
        in_pool = ctx.enter_context(tc.tile_pool(name="in", bufs=6))
        sq_pool = ctx.enter_context(tc.tile_pool(name="sq", bufs=3))
        psum_pool = ctx.enter_context(tc.tile_pool(name="psum", bufs=1, space="PSUM"))
        const_pool = ctx.enter_context(tc.tile_pool(name="const", bufs=1))
        res_pool = ctx.enter_context(tc.tile_pool(name="res", bufs=1))

        ones = const_pool.tile([P, 1], BF16)
        nc.vector.memset(ones, 1.0)
        psum = psum_pool.tile([1, N_COLS], mybir.dt.float32)

        for s in range(S):
            t_in = in_pool.tile([P, T, N_COLS], BF16)
            # alternate the two HWDGE rings so per-DMA fixed costs overlap
            dma_eng = nc.sync if s % 2 == 0 else nc.scalar
            dma_eng.dma_start(out=t_in, in_=view[s])
            sq = sq_pool.tile([P, T, N_COLS], BF16)
            # square on DVE (bf16 in/out -> 2x mode) in halves so the matmuls
            # of the first half overlap the second half's square
            H = T // 2
            for h in range(2):
                hs = slice(h * H, (h + 1) * H)
                nc.vector.tensor_mul(sq[:, hs, :], t_in[:, hs, :], t_in[:, hs, :])
                for t in range(h * H, (h + 1) * H):
                    # psum[1, 512] += ones[128, 1].T @ sq[:, t, :]
                    nc.tensor.matmul(
                        psum,
                        lhsT=ones,
                        rhs=sq[:, t, :],
                        start=(s == 0 and t == 0),
                        stop=(s == S - 1 and t == T - 1),
                    )

        res = res_pool.tile([1, N_COLS], mybir.dt.float32)
        nc.vector.tensor_copy(res, psum)
        nc.sync.dma_start(out=out, in_=res)

    nc.compile()
    return nc


def _run_device(D, **spmd_kwargs):
    """Run the per-core partial reduction; returns (partials [8, 512], results)."""
    if "nc" not in _NC_CACHE:
        _NC_CACHE["nc"] = _build_nc()
    nc = _NC_CACHE["nc"]
    D16 = np.asarray(D, dtype=np.float32).astype(ml_dtypes.bfloat16)
    shards = np.split(D16, N_CORES, axis=0)
    in_maps = [{"d_shard": np.ascontiguousarray(s)} for s in shards]
    res = run_bass_kernel_spmd(nc, in_maps, core_ids=list(range(N_CORES)), **spmd_kwargs)
    partials = np.stack([np.asarray(r["partial"]).reshape(N_COLS) for r in res.results])
    return partials, res


def kernel(D):
    partials, _ = _run_device(D)
    total = partials.sum(axis=0, dtype=np.float64)
    resid = total - 1.0
    loss = 0.001 * np.sqrt(np.sum(resid * resid))
    return np.array(loss, dtype=np.float32)
